# revision 5
# baseline (speedup 1.0000x reference)
"""AttentionBlock (GroupNorm + single-head self-attention + residual) on 8 TRN2
NeuronCores, data-parallel over the batch dim (B=8, one batch element per core).

Key structure (all big matmuls fp8 DoubleRow, 256-deep contraction, N=512):
  - Wq folded into the k-projection (bq==0 path): S^T = h^T (Wq^T Wk) h, so
    W_eff = Wq^T Wk * SCALE_W/sqrt(C) is precomputed on host (SCALE_W keeps
    the product in fp8 range; 1/SCALE_W folds into the exp scale) and the
    entire q-projection disappears — score matmuls consume h directly.
    A bq != 0 input falls back to the explicit q-projection path.
  - x resident in SBUF fp32 (no bf16 copy pass, no residual re-load).
  - 4096/2048-wide phase-1 stats; rstd = exp(-0.5*ln(var+eps)) keeps the ACT
    table set pinned to ln/exp (no per-iteration table reloads).
  - score PSUM tiles hold a kt-PAIR ([128, 2, 512] fp32, 2 banks); softmax exp
    is one 1024-wide ACT instruction per pair.
  - k-bias dropped: S^T[k,q] shifts by (bk.q)[q], constant along the softmax
    axis, which cancels exactly between numerator and denominator.
  - softmax denominator accumulated on the otherwise-idle Pool engine (even
    pairs) and DVE (odd pairs), then 4 small ones-matmuls; the 128 per-pair
    ones-matmuls of the baseline are gone from the PE.
  - k/v/O PSUM drains pair-packed into 1024-wide instructions, split ACT/DVE.
  - dummy matmuls (gated on the stats results) keep the PE HAM clock warm
    through the serial stats chain.
"""

import numpy as np
import ml_dtypes
from contextlib import ExitStack

import concourse.bass as bass
import concourse.tile as tile
from concourse import bacc, mybir
from concourse.bass_utils import run_bass_kernel_spmd

C = 512
GROUPS = 32
EPS = 1e-6
CT = C // 128          # 4 channel tiles of 128
CHUNK = 512            # q-chunk width (one PSUM bank of fp32)
F32 = mybir.dt.float32
BF16 = mybir.dt.bfloat16
FP8 = mybir.dt.float8e4
DR = mybir.MatmulPerfMode.DoubleRow
AF = mybir.ActivationFunctionType
ALU = mybir.AluOpType
AX = mybir.AxisListType

GPC = C // GROUPS      # channels per group = 16
GPT = 128 // GPC       # groups per channel-tile = 8


SCALE_W = 512.0


def build_nc(n_pix=4096, repeat=1, fold_q=True):
    """repeat>1 wraps the whole body in a hardware loop — used only for timing
    (amortizes the ~80ms per-call axon dispatch overhead over R executions)."""
    nt = n_pix // 128          # number of 128-wide pixel tiles (k tiles)
    nchunk = n_pix // CHUNK    # number of q chunks
    inv_cnt = 1.0 / (GPC * n_pix)
    # with fold_q the 1/sqrt(C) is folded into the host-side W_eff = Wq^T Wk
    # (rescaled by SCALE_W to stay in fp8 range)
    scale_s = (1.0 / SCALE_W) if fold_q else 1.0 / float(np.sqrt(C))

    nc = bacc.Bacc(trn_type="TRN2", target_bir_lowering=False, debug=False)

    xd = nc.declare_dram_parameter("x", [C, n_pix], F32, isOutput=False)
    wqd = (None if fold_q else nc.declare_dram_parameter(
        "wqT2", [CT // 2, 128, 2, C], FP8, isOutput=False))
    wkd = nc.declare_dram_parameter("wkT2", [CT // 2, 128, 2, C], FP8, isOutput=False)
    wvd = nc.declare_dram_parameter("wvT2", [CT // 2, 128, 2, C], FP8, isOutput=False)
    wod = nc.declare_dram_parameter("woT2", [CT // 2, 128, 2, C], FP8, isOutput=False)
    # per-channel vectors packed [128, CT]: column ct = channels ct*128..+128
    gsd = nc.declare_dram_parameter("gn_scale", [128, CT], F32, isOutput=False)
    gbd = nc.declare_dram_parameter("gn_bias", [128, CT], F32, isOutput=False)
    bqd = (None if fold_q else nc.declare_dram_parameter(
        "bq", [128, CT], F32, isOutput=False))
    bod = nc.declare_dram_parameter("bo", [128, CT], F32, isOutput=False)
    outd = nc.declare_dram_parameter("out", [C, n_pix], F32, isOutput=True)

    gmat_np = np.zeros((128, GPT), np.float32)
    for p in range(128):
        gmat_np[p, p // GPC] = 1.0
    gmat_d = nc.inline_tensor(gmat_np, name="gmat")
    gmat_t_d = nc.inline_tensor(np.ascontiguousarray(gmat_np.T), name="gmat_t")
    ones_col_b_d = nc.inline_tensor(
        np.ones((128, 1), ml_dtypes.bfloat16), name="ones_col_b")
    ones_row_b_d = nc.inline_tensor(
        np.ones((1, 128), ml_dtypes.bfloat16), name="ones_row_b"
    )

    with tile.TileContext(nc) as tc, ExitStack() as ctx:
        cp = ctx.enter_context(tc.tile_pool(name="consts", bufs=1))
        res = ctx.enter_context(tc.tile_pool(name="res", bufs=1))
        hp = ctx.enter_context(tc.tile_pool(name="hp", bufs=10))
        scr = ctx.enter_context(tc.tile_pool(name="scr", bufs=2))
        qp = ctx.enter_context(tc.tile_pool(name="qp", bufs=10))
        ptp = ctx.enter_context(tc.tile_pool(name="ptp", bufs=8))
        dnp = ctx.enter_context(tc.tile_pool(name="dnp", bufs=2))
        rbp = ctx.enter_context(tc.tile_pool(name="rbp", bufs=2))
        oup = ctx.enter_context(tc.tile_pool(name="oup", bufs=6))
        ep = ctx.enter_context(tc.tile_pool(name="ep", bufs=6))
        psS = ctx.enter_context(tc.tile_pool(name="psS", bufs=2, space="PSUM"))
        psO = ctx.enter_context(tc.tile_pool(name="psO", bufs=2, space="PSUM"))

        if repeat > 1:
            loop_cm = tc.For_i(0, repeat, hint_engines=(
                mybir.EngineType.PE, mybir.EngineType.Activation,
                mybir.EngineType.DVE, mybir.EngineType.SP,
                mybir.EngineType.Pool))
            loop_cm.__enter__()

        # ---- phase 1: load x (resident fp32), per-group stats ----
        x_res = [res.tile([128, n_pix], F32, name=f"x{ct}", tag=f"x{ct}")
                 for ct in range(CT)]
        for ct in range(CT):
            nc.sync.dma_start(x_res[ct][:], xd.ap()[ct * 128:(ct + 1) * 128, :])

        def load_vec(dram, label):
            t = cp.tile([128, CT], F32, name=label, tag=label)
            nc.sync.dma_start(t[:], dram.ap())
            return t

        gs_all = load_vec(gsd, "gs_all")
        gb_all = load_vec(gbd, "gb_all")
        gmat = cp.tile([128, GPT], F32, name="gmat_sb", tag="gmat")
        nc.sync.dma_start(gmat[:], gmat_d.ap())
        gmat_t = cp.tile([GPT, 128], F32, name="gmatT_sb", tag="gmatT")
        nc.sync.dma_start(gmat_t[:], gmat_t_d.ap())

        # stats_all col layout per ct: [3ct] = sum(x), [3ct+1], [3ct+2] = two
        # half-accumulated sum(x^2) pieces
        stats_all = cp.tile([128, 3 * CT], F32, name="stats_all", tag="stats_all")
        half = n_pix // 2
        for ct in range(CT):
            nc.vector.reduce_sum(stats_all[:, 3 * ct:3 * ct + 1], x_res[ct][:],
                                 axis=AX.X)
            for h in range(2):
                sq = scr.tile([128, half], F32, name=f"sq{ct}_{h}", tag="sq")
                nc.scalar.activation(
                    sq[:], x_res[ct][:, h * half:(h + 1) * half], AF.Square,
                    accum_out=stats_all[:, 3 * ct + 1 + h:3 * ct + 2 + h])

        # remaining constants/vectors (not stats-critical)
        ones_col_b = cp.tile([128, 1], BF16, name="ones_col_b_sb", tag="ones_col_b")
        nc.sync.dma_start(ones_col_b[:], ones_col_b_d.ap())
        ones_row_b = cp.tile([1, 128], BF16, name="ones_row_b_sb", tag="ones_row_b")
        nc.sync.dma_start(ones_row_b[:], ones_row_b_d.ap())
        bq_all = None if fold_q else load_vec(bqd, "bq_all")
        bo_all = load_vec(bod, "bo_all")

        def load_w(dram, label):
            ws = []
            for p in range(CT // 2):
                t = res.tile([128, 2, C], FP8, name=f"{label}{p}", tag=f"{label}{p}")
                nc.sync.dma_start(t[:], dram.ap()[p])
                ws.append(t)
            return ws

        # weights loaded after x so the stats-critical x DMAs go first on the ring
        wk_bf = load_w(wkd, "wk")
        wv_bf = load_w(wvd, "wv")
        wq_bf = None if fold_q else load_w(wqd, "wq")
        wo_bf = load_w(wod, "wo")

        # ---- resident tensors ----
        k2 = [res.tile([128, 2, n_pix], FP8, name=f"k2_{p}", tag=f"k2_{p}")
              for p in range(CT // 2)]
        vT2 = [res.tile([128, 2, C], FP8, name=f"vT2_{i}", tag=f"vT2_{i}")
               for i in range(nt // 2)]
        # warm-up operand for HAM dummy matmuls
        wtile = res.tile([128, CHUNK], BF16, name="wtile", tag="wtile")
        nc.vector.memset(wtile[:], 0.0)

        # one matmul for all cross-partition group sums: [128, 12] -> [8, 12]
        pg = psO.tile([GPT, 3 * CT], F32, name="pg", tag="po")
        nc.tensor.matmul(pg[:], lhsT=gmat[:], rhs=stats_all[:], start=True, stop=True)
        gsb = cp.tile([GPT, 3 * CT], F32, name="gsb", tag="gsb")
        nc.scalar.copy(gsb[:], pg[:])

        # dummy matmuls gated on gsb keep the PE busy (HAM warm) through the
        # serial stats chain below
        nc.vector.tensor_copy(wtile[0:GPT, 0:3 * CT], gsb[:])
        warm = psS.tile([128, CHUNK], F32, name="warm", tag="ps")
        for i in range(10):
            nc.tensor.matmul(warm[:], lhsT=wtile[:, 0:128], rhs=wtile[:],
                             start=(i == 0), stop=(i == 9))

        mu44 = cp.tile([GPT, CT], F32, name="mu44", tag="mu44")
        ex2 = cp.tile([GPT, CT], F32, name="ex2", tag="ex2")
        musq = cp.tile([GPT, CT], F32, name="musq", tag="musq")
        var44 = cp.tile([GPT, CT], F32, name="var44", tag="var44")
        vare = cp.tile([GPT, CT], F32, name="vare", tag="vare")
        lnv = cp.tile([GPT, CT], F32, name="lnv", tag="lnv")
        rstd44 = cp.tile([GPT, CT], F32, name="rstd44", tag="rstd44")
        mr = cp.tile([GPT, 2 * CT], F32, name="mr", tag="mr")
        nc.scalar.mul(mu44[:], gsb[0:GPT, 0:3 * CT:3], inv_cnt)
        nc.vector.tensor_add(ex2[:], gsb[0:GPT, 1:3 * CT:3],
                             gsb[0:GPT, 2:3 * CT:3])
        nc.vector.tensor_scalar_mul(ex2[:], ex2[:], inv_cnt)
        nc.vector.tensor_mul(musq[:], mu44[:], mu44[:])
        nc.vector.tensor_sub(var44[:], ex2[:], musq[:])
        nc.vector.tensor_scalar_add(vare[:], var44[:], EPS)
        # rstd = exp(-0.5 * ln(var+eps)) — stays inside the ln/exp ACT table set
        nc.scalar.activation(lnv[:], vare[:], AF.Ln)
        nc.scalar.activation(rstd44[:], lnv[:], AF.Exp, scale=-0.5)
        nc.vector.tensor_copy(mr[0:GPT, 0:2 * CT:2], mu44[:])
        nc.vector.tensor_copy(mr[0:GPT, 1:2 * CT:2], rstd44[:])

        # broadcast group mu/rstd back to channels: [8, 8] -> [128, 8]
        pmc = psO.tile([128, 2 * CT], F32, name="pmc", tag="po")
        nc.tensor.matmul(pmc[:], lhsT=gmat_t[:], rhs=mr[:], start=True, stop=True)
        mcall = cp.tile([128, 2 * CT], F32, name="mcall", tag="mcall")
        nc.scalar.copy(mcall[:], pmc[:])
        a_all = cp.tile([128, CT], F32, name="a_all", tag="a_all")
        nc.vector.tensor_mul(a_all[:], mcall[:, 1:2 * CT:2], gs_all[:])
        btmp = cp.tile([128, CT], F32, name="btmp", tag="btmp")
        nc.vector.tensor_mul(btmp[:], mcall[:, 0:2 * CT:2], a_all[:])
        b_all = cp.tile([128, CT], F32, name="b_all", tag="b_all")
        nc.vector.tensor_sub(b_all[:], gb_all[:], btmp[:])

        # ---- phase 2: h = a*x + b per chunk (fp8); k and vT projections ----
        def h_chunk(j, uid, dve_only=False):
            """Cast one 512-wide chunk of h (fp8 pair layout) for all 4 channel
            tiles, reading resident fp32 x. ACT/DVE split unless dve_only."""
            cols = slice(j * CHUNK, (j + 1) * CHUNK)
            hs = [hp.tile([128, 2, CHUNK], FP8, name=f"h{uid}_{j}_{p}", tag="h")
                  for p in range(CT // 2)]
            for ct in range(CT):
                hdst = hs[ct // 2][:, ct % 2, :]
                if ct % 2 == 0 and not dve_only:
                    nc.scalar.activation(hdst, x_res[ct][:, cols], AF.Identity,
                                         scale=a_all[:, ct:ct + 1],
                                         bias=b_all[:, ct:ct + 1])
                else:
                    nc.vector.tensor_scalar(hdst, x_res[ct][:, cols],
                                            a_all[:, ct:ct + 1],
                                            b_all[:, ct:ct + 1],
                                            op0=ALU.mult, op1=ALU.add)
            return hs

        for j in range(nchunk):
            cols = slice(j * CHUNK, (j + 1) * CHUNK)
            hs = h_chunk(j, "p2")  # fp8 pair tiles for this chunk
            # k projection: paired PSUM [128, 2, 512], one 1024-wide drain.
            # No k-bias: softmax is exactly invariant to it (see module doc).
            for p in range(CT // 2):
                pk2 = psO.tile([128, 2, CHUNK], F32, name=f"pk{p}_{j}", tag="po")
                for ch2 in range(2):
                    ct = 2 * p + ch2
                    for pw in range(CT // 2):
                        nc.tensor.matmul(pk2[:, ch2, :],
                                         lhsT=wk_bf[pw][:, :, ct * 128:(ct + 1) * 128],
                                         rhs=hs[pw][:],
                                         start=(pw == 0), stop=(pw == CT // 2 - 1),
                                         perf_mode=DR)
                kdst = k2[p][:, :, cols]
                if p == 0:
                    nc.scalar.copy(kdst, pk2[:])
                else:
                    nc.vector.tensor_copy(kdst, pk2[:])
            # v projection: paired over pixel sub-tiles, one 1024-wide drain
            for ip in range(2):
                i0 = 4 * j + 2 * ip
                pv2 = psO.tile([128, 2, C], F32, name=f"pv{ip}_{j}", tag="po")
                for ch2 in range(2):
                    off = (2 * ip + ch2) * 128
                    for pw in range(CT // 2):
                        nc.tensor.matmul(pv2[:, ch2, :],
                                         lhsT=hs[pw][:, :, off:off + 128],
                                         rhs=wv_bf[pw][:],
                                         start=(pw == 0), stop=(pw == CT // 2 - 1),
                                         perf_mode=DR)
                vdst = vT2[i0 // 2][:]
                if ip == 0:
                    nc.vector.tensor_copy(vdst, pv2[:])
                else:
                    nc.scalar.copy(vdst, pv2[:])

        # ---- phase 3: attention, one q-chunk at a time ----
        def q_proj(ch):
            hs = h_chunk(ch, "q")
            if fold_q:
                return hs
            qs = [qp.tile([128, 2, CHUNK], FP8, name=f"qs{ch}_{p}", tag="qs")
                  for p in range(CT // 2)]
            for ct in range(CT):
                pq = psS.tile([128, CHUNK], F32, name=f"pq{ch}_{ct}", tag="ps")
                for p in range(CT // 2):
                    nc.tensor.matmul(pq[:],
                                     lhsT=wq_bf[p][:, :, ct * 128:(ct + 1) * 128],
                                     rhs=hs[p][:],
                                     start=(p == 0), stop=(p == CT // 2 - 1),
                                     perf_mode=DR)
                qdst = qs[ct // 2][:, ct % 2, :]
                nc.vector.tensor_scalar_add(qdst, pq[:], bq_all[:, ct:ct + 1])
            return qs

        qs = q_proj(0)
        for ch in range(nchunk):
            cols = slice(ch * CHUNK, (ch + 1) * CHUNK)
            po2 = [psO.tile([128, 2, CHUNK], F32, name=f"po{ch}_{j}", tag="po")
                   for j in range(2)]
            npair = nt // 2
            pts = [None] * npair
            # softmax denominator partial accumulators: Pool takes even pairs,
            # DVE odd pairs; 4 ones-matmuls fold both + the pair axis at the end
            dn_p = dnp.tile([128, 2, CHUNK], BF16, name=f"dnp{ch}", tag="dnp")
            dn_d = dnp.tile([128, 2, CHUNK], BF16, name=f"dnd{ch}", tag="dnd")

            def o_pair(pp):
                for ct in range(CT):
                    nc.tensor.matmul(po2[ct // 2][:, ct % 2, :],
                                     lhsT=vT2[pp][:, :, ct * 128:(ct + 1) * 128],
                                     rhs=pts[pp][:],
                                     start=(pp == 0), stop=(pp == npair - 1),
                                     perf_mode=DR)

            for pp in range(npair):
                ps2 = psS.tile([128, 2, CHUNK], F32, name=f"ps{ch}_{pp}", tag="ps")
                for h2 in range(2):
                    kt = 2 * pp + h2
                    for p in range(CT // 2):
                        nc.tensor.matmul(ps2[:, h2, :],
                                         lhsT=k2[p][:, :, kt * 128:(kt + 1) * 128],
                                         rhs=qs[p][:],
                                         start=(p == 0), stop=(p == CT // 2 - 1),
                                         perf_mode=DR)
                pts[pp] = ptp.tile([128, 2, CHUNK], FP8, name=f"pt{ch}_{pp}",
                                   tag="pt")
                # one 1024-wide exp per kt pair
                nc.scalar.activation(pts[pp][:], ps2[:], AF.Exp, scale=scale_s)
                # denominator accumulation rides on Pool (even) / DVE (odd);
                # the last DVE add is deferred past the ou drain so the
                # epilogue's pz matmuls are not queued behind it
                if pp < npair - 1:
                    eng = nc.gpsimd if pp % 2 == 0 else nc.vector
                    dst = dn_p if pp % 2 == 0 else dn_d
                    if pp < 2:
                        eng.tensor_copy(dst[:], pts[pp][:])
                    else:
                        eng.tensor_add(dst[:], dst[:], pts[pp][:])
                # O matmuls lag one completed pair (keeps PE off the ACT path)
                if pp >= 1:
                    o_pair(pp - 1)
            o_pair(npair - 1)

            # unnormalized O -> fp8 pair sbuf, one 1024-wide drain per half
            ou = [oup.tile([128, 2, CHUNK], FP8, name=f"ou{ch}_{p}", tag="ou")
                  for p in range(CT // 2)]
            nc.vector.tensor_copy(ou[1][:], po2[1][:])
            nc.scalar.copy(ou[0][:], po2[0][:])
            nc.vector.tensor_add(dn_d[:], dn_d[:], pts[npair - 1][:])

            # next chunk's q projection fills the PE while the denominator
            # chain resolves
            if ch + 1 < nchunk:
                qs_next = q_proj(ch + 1)

            # cross-partition + cross-half + cross-engine denominator fold:
            # [1, 512] = sum over 128 partitions of the 4 bf16 partials
            pd2 = psS.tile([1, CHUNK], F32, name=f"pd2_{ch}", tag="ps")
            mmi = 0
            for dst in (dn_p, dn_d):
                for h2 in range(2):
                    nc.tensor.matmul(pd2[:], lhsT=ones_col_b[:],
                                     rhs=dst[:, h2, :],
                                     start=(mmi == 0), stop=(mmi == 3))
                    mmi += 1
            r32 = rbp.tile([1, CHUNK], F32, name=f"r32_{ch}", tag="r32")
            nc.vector.reciprocal(r32[:], pd2[:])
            rbf = rbp.tile([1, CHUNK], BF16, name=f"rbf{ch}", tag="rbf")
            nc.vector.tensor_copy(rbf[:], r32[:])

            # output projection + deferred normalization + residual + bo;
            # the prb broadcast matmul is emitted after the pz matmuls so the
            # PE never waits on the DVE reciprocal chain
            pz2s = []
            for jp in range(2):
                pz2 = psO.tile([128, 2, CHUNK], F32, name=f"pz{ch}_{jp}", tag="po")
                for ch2 in range(2):
                    oct = 2 * jp + ch2
                    for p in range(CT // 2):
                        nc.tensor.matmul(pz2[:, ch2, :],
                                         lhsT=wo_bf[p][:, :, oct * 128:(oct + 1) * 128],
                                         rhs=ou[p][:],
                                         start=(p == 0), stop=(p == CT // 2 - 1),
                                         perf_mode=DR)
                pz2s.append(pz2)
            prb = psS.tile([128, CHUNK], F32, name=f"prb{ch}", tag="ps")
            nc.tensor.matmul(prb[:], lhsT=ones_row_b[:], rhs=rbf[:],
                             start=True, stop=True)
            rb = rbp.tile([128, CHUNK], F32, name=f"rb{ch}", tag="rb")
            nc.vector.tensor_copy(rb[:], prb[:])
            for oct in range(CT):
                t1 = ep.tile([128, CHUNK], F32, name=f"t1_{ch}_{oct}", tag="t1")
                nc.vector.tensor_mul(t1[:], pz2s[oct // 2][:, oct % 2, :], rb[:])
                osb = ep.tile([128, CHUNK], F32, name=f"osb{ch}_{oct}",
                              tag="osb")
                nc.vector.scalar_tensor_tensor(osb[:], t1[:],
                                               bo_all[:, oct:oct + 1],
                                               x_res[oct][:, cols],
                                               op0=ALU.add, op1=ALU.add)
                nc.sync.dma_start(outd.ap()[oct * 128:(oct + 1) * 128, cols],
                                  osb[:])

            if ch + 1 < nchunk:
                qs = qs_next

        if repeat > 1:
            loop_cm.__exit__(None, None, None)

    nc.compile()
    return nc


_NC_CACHE = {}


def _get_nc(n_pix, fold_q=True):
    key = (n_pix, fold_q)
    if key not in _NC_CACHE:
        _NC_CACHE[key] = build_nc(n_pix, fold_q=fold_q)
    return _NC_CACHE[key]


def make_in_maps(x, gn_scale, gn_bias, Wq, bq, Wk, bk, Wv, bv, Wo, bo):
    B, C_, H, W = x.shape
    n_pix = H * W

    def vec(v):
        return np.ascontiguousarray(
            np.asarray(v, np.float32).reshape(CT, 128).T)

    def wT2(w):
        """wT [C, C] -> pair-packed [CT//2, 128, 2, C] fp8 (DoubleRow layout)."""
        wt = np.asarray(w, np.float32).T.reshape(CT // 2, 2, 128, C)
        return np.ascontiguousarray(
            wt.transpose(0, 2, 1, 3).astype(ml_dtypes.float8_e4m3))

    # v-bias folds into the output bias: softmax rows sum to 1, so
    # out = x + Wo @ (v_0 P^T / denom) + (bo + Wo @ bv).
    # k-bias is dropped entirely (softmax shift invariance along k).
    # With bq == 0 (fold_q), Wq folds into the k-projection:
    # S^T = h^T (Wq^T Wk) h, so W_eff = Wq^T Wk * SCALE_W / sqrt(C) replaces
    # Wk and the q-projection disappears (scores consume h directly).
    fold_q = not np.any(np.asarray(bq))
    bo_eff = (np.asarray(bo, np.float64)
              + np.asarray(Wo, np.float64) @ np.asarray(bv, np.float64))
    if fold_q:
        w_eff = (np.asarray(Wq, np.float64).T @ np.asarray(Wk, np.float64)
                 ) * (SCALE_W / np.sqrt(C))
        base = {
            "wkT2": wT2(w_eff),
            "wvT2": wT2(Wv),
            "woT2": wT2(Wo),
            "gn_scale": vec(gn_scale),
            "gn_bias": vec(gn_bias),
            "bo": vec(bo_eff),
        }
    else:
        base = {
            "wqT2": wT2(Wq),
            "wkT2": wT2(Wk),
            "wvT2": wT2(Wv),
            "woT2": wT2(Wo),
            "gn_scale": vec(gn_scale),
            "gn_bias": vec(gn_bias),
            "bq": vec(bq),
            "bo": vec(bo_eff),
        }
    f32 = lambda v: np.ascontiguousarray(np.asarray(v, np.float32))
    return [dict(base, x=f32(np.asarray(x[b], np.float32).reshape(C_, n_pix)))
            for b in range(B)]


def kernel(x, gn_scale, gn_bias, Wq, bq, Wk, bk, Wv, bv, Wo, bo):
    x = np.asarray(x)
    B, C_, H, W = x.shape
    n_pix = H * W
    fold_q = not np.any(np.asarray(bq))
    nc = _get_nc(n_pix, fold_q)
    in_maps = make_in_maps(x, gn_scale, gn_bias, Wq, bq, Wk, bk, Wv, bv, Wo, bo)
    res = run_bass_kernel_spmd(nc, in_maps, core_ids=list(range(B)))
    out = np.stack([res.results[b]["out"] for b in range(B)])
    return out.reshape(B, C_, H, W).astype(np.float32)
